# revision 16
# baseline (speedup 1.0000x reference)
"""Distributed k-NN retrieval (MemoryBank) on 8 Trainium2 NeuronCores.

Strategy (memory rows sharded 8 ways, queries replicated):
  Device (per core):
    - normalize its memory shard rows (1/max(|m|,eps)), cast bf16,
      DMA-transpose to [D, M] layout; cast+transpose queries (not normalized:
      a per-query positive scale never changes that query's ranking).
    - 32 query tiles x 26 matmul chunks (N=512) -> PSUM f32 sims.
    - max-accumulate drain: tensor_tensor(max)(acc, psum, acc) folds each
      chunk into a [128, 1024] bf16 accumulator (group u = stride-1024 family
      of 13 memory rows). One PSUM read per sims element - the hard floor.
    - ship [4096, 1024] bf16 group-max matrix per core to host.
  Host:
    - top-6 groups per query across all cores (top-3 groups provably contain
      the true top-3 values), rescore <=78 candidate rows exactly in fp32,
      emit top-k (distances = 1-sims, indices), ties -> lowest index.
"""

import functools

import numpy as np

# ---- hardcoded problem geometry (self-contained; do not read spec files) ----
NQ = 4096          # queries
D = 128            # feature dim
M_TOTAL = 100000   # memory rows
N_CORES = 8
M_SHARD = 13312    # padded per-core rows = 104*128 = 13*1024
M_PAD_TOTAL = M_SHARD * N_CORES
NQT = NQ // 128    # 32 query tiles
N_CHUNK = 26       # matmul chunks of 512 per query tile
CM_W = 1024        # group-max width; group u = rows {1024*i + u}, 13 members
N_MTILE = M_SHARD // 128  # 104
EPS = 1e-12

# number of top groups rescored on host (3 suffices in exact arithmetic;
# extra groups absorb bf16 rounding ties)
T_GROUPS = 6

# knob: chunks whose PSUM halves are drained by ScalarE copies (DVE then pairs
# them in SBUF at 2x) instead of the DVE reading PSUM directly. Balances the
# two engines; tune against hardware profile.
ACT_CHUNKS = frozenset()


@functools.lru_cache(maxsize=1)
def _build_nc():
    import concourse.mybir as mybir
    from concourse import bacc, tile

    f32 = mybir.dt.float32
    bf16 = mybir.dt.bfloat16
    AF = mybir.ActivationFunctionType
    MAX = mybir.AluOpType.max
    AX = mybir.AxisListType.X

    nc = bacc.Bacc("TRN2", target_bir_lowering=False, debug=False)

    mem_in = nc.dram_tensor("mem", [M_SHARD, D], f32, kind="ExternalInput")
    q_in = nc.dram_tensor("queries", [NQ, D], f32, kind="ExternalInput")
    id_in = nc.dram_tensor("ident", [128, 128], bf16, kind="ExternalInput")
    cm_out = nc.dram_tensor("cm", [NQ, CM_W], bf16, kind="ExternalOutput")

    with tile.TileContext(nc) as tc:
        with (
            tc.tile_pool(name="const", bufs=1) as const_pool,
            tc.tile_pool(name="stage", bufs=1) as stage_pool,
            tc.tile_pool(name="prep", bufs=2) as prep_pool,
            tc.tile_pool(name="psum", bufs=4, space="PSUM") as psum_pool,
            tc.tile_pool(name="tpsum", bufs=4, space="PSUM") as tpsum_pool,
            tc.tile_pool(name="work", bufs=2) as work_pool,
        ):
            mT = const_pool.tile([128, M_SHARD], bf16, tag="mT")
            qT = const_pool.tile([128, NQ], bf16, tag="qT")
            ident = const_pool.tile([128, 128], bf16, tag="ident")
            nc.sync.dma_start(ident[:], id_in.ap())

            # ---------------- prep: queries -> qT (bf16, transposed) --------
            qstage = stage_pool.tile([128, NQT * D], f32, tag="qstage")
            nc.sync.dma_start(
                qstage[:].rearrange("p (t d) -> p t d", d=D),
                q_in.ap().rearrange("(t p) d -> p t d", p=128),
            )
            q_bf = prep_pool.tile([128, NQT * D], bf16, tag="q_bf")
            for t in range(NQT):
                nc.scalar.activation(
                    q_bf[:, t * D:(t + 1) * D], qstage[:, t * D:(t + 1) * D],
                    AF.Copy,
                )
            for t in range(NQT):
                tp = tpsum_pool.tile([128, 128], bf16, tag="tp")
                nc.tensor.transpose(tp[:], q_bf[:, t * D:(t + 1) * D], ident[:])
                nc.vector.tensor_copy(qT[:, t * 128:(t + 1) * 128], tp[:])

            # ---------------- prep: memory -> mT (normalized bf16, transposed)
            ss = const_pool.tile([128, N_MTILE], f32, tag="ss")
            N_PIECE = 4
            TPP = N_MTILE // N_PIECE  # 26 tiles per piece
            for piece in range(N_PIECE):
                mstage = stage_pool.tile([128, TPP * D], f32, tag=f"mstage{piece}")
                r0 = piece * TPP * 128
                nc.sync.dma_start(
                    mstage[:].rearrange("p (t d) -> p t d", d=D),
                    mem_in.ap()[r0:r0 + TPP * 128, :].rearrange(
                        "(t p) d -> p t d", p=128),
                )
                sq = prep_pool.tile([128, TPP * D], f32, tag="sq")
                nc.scalar.activation(sq[:], mstage[:], AF.Square)
                nc.vector.reduce_sum(
                    ss[:, piece * TPP:(piece + 1) * TPP],
                    sq[:].rearrange("p (t d) -> p t d", d=D),
                    axis=AX,
                )
                # normalized bf16 tiles + transpose into mT
                m_bf = prep_pool.tile([128, TPP * D], bf16, tag="m_bf")
                norm = prep_pool.tile([128, TPP], f32, tag="norm")
                scale = prep_pool.tile([128, TPP], f32, tag="scale")
                nc.scalar.activation(
                    norm[:], ss[:, piece * TPP:(piece + 1) * TPP], AF.Sqrt)
                nc.vector.tensor_scalar_max(norm[:], norm[:], EPS)
                nc.vector.reciprocal(scale[:], norm[:])
                for t in range(TPP):
                    nc.scalar.activation(
                        m_bf[:, t * D:(t + 1) * D],
                        mstage[:, t * D:(t + 1) * D],
                        AF.Copy,
                        scale=scale[:, t:t + 1],
                    )
                for t in range(TPP):
                    tg = piece * TPP + t
                    tp = tpsum_pool.tile([128, 128], bf16, tag="tp")
                    nc.tensor.transpose(tp[:], m_bf[:, t * D:(t + 1) * D], ident[:])
                    nc.vector.tensor_copy(mT[:, tg * 128:(tg + 1) * 128], tp[:])

            # ---------------- main: sims + max-accumulate drain -------------
            # acc[:, u] ends as max over i of sims[:, 1024*i + u]
            # (chunk c of 512 lands in acc half c%2; group = stride-1024 family)
            for qt in range(NQT):
                acc = work_pool.tile([128, CM_W], bf16, tag="acc")
                lhsT = qT[:, qt * 128:(qt + 1) * 128]
                for c in range(N_CHUNK):
                    ps = psum_pool.tile([128, 512], f32, tag="ps")
                    nc.tensor.matmul(
                        ps[:], lhsT, mT[:, c * 512:(c + 1) * 512],
                        start=True, stop=True,
                    )
                    dst = acc[:, (c % 2) * 512:(c % 2) * 512 + 512]
                    if c < 2:
                        nc.vector.tensor_copy(dst, ps[:])
                    elif c in ACT_CHUNKS:
                        tmp = work_pool.tile([128, 512], bf16, tag="tmp")
                        nc.scalar.copy(tmp[:], ps[:])
                        nc.vector.tensor_tensor(dst, tmp[:], dst, op=MAX)
                    else:
                        nc.vector.tensor_tensor(dst, ps[:], dst, op=MAX)
                nc.sync.dma_start(cm_out.ap()[qt * 128:(qt + 1) * 128, :], acc[:])

    nc.compile()
    return nc


def _identity_bf16():
    import ml_dtypes

    return np.eye(128, dtype=ml_dtypes.bfloat16)


def _in_maps(queries_np, mem_padded):
    shards = mem_padded.reshape(N_CORES, M_SHARD, D)
    ident = _identity_bf16()
    return [
        {"mem": np.ascontiguousarray(shards[c]), "queries": queries_np,
         "ident": ident}
        for c in range(N_CORES)
    ]


def _run_device(queries_np, mem_padded, trace=False):
    from concourse import bass_utils

    nc = _build_nc()
    res = bass_utils.run_bass_kernel_spmd(
        nc, _in_maps(queries_np, mem_padded),
        core_ids=list(range(N_CORES)), trace=trace,
    )
    return res


def _host_topk(queries_np, memory_np, cm_all, k):
    import ml_dtypes  # noqa: F401  (cm arrives as bfloat16)

    nq = queries_np.shape[0]
    # [NQ, N_CORES*CM_W] group-max matrix
    cm = np.concatenate(
        [np.asarray(cm_all[c], dtype=np.float32) for c in range(N_CORES)], axis=1
    )
    t = min(T_GROUPS, cm.shape[1])
    top_groups = np.argpartition(-cm, t - 1, axis=1)[:, :t]  # [NQ, t]

    core = top_groups // CM_W
    g = top_groups % CM_W
    # group (core, u) covers local rows {1024*i + u : i < M_SHARD // 1024}
    i = np.arange(M_SHARD // CM_W)
    loc = g[:, :, None] + CM_W * i[None, None, :]         # [NQ, t, 13]
    cand = (core[:, :, None] * M_SHARD + loc).reshape(nq, -1)  # [NQ, t*13]

    valid = cand < M_TOTAL
    cand_safe = np.where(valid, cand, 0)

    qn = queries_np / np.maximum(
        np.linalg.norm(queries_np, axis=1, keepdims=True), EPS)
    mc = memory_np[cand_safe]                             # [NQ, t*16, D]
    mc_n = np.linalg.norm(mc, axis=2, keepdims=True)
    mc = mc / np.maximum(mc_n, EPS)
    vals = np.einsum("qd,qcd->qc", qn.astype(np.float32), mc.astype(np.float32))
    vals = np.where(valid, vals, np.float32(-2.0))

    # sort candidates by index so a stable sort on -vals breaks ties by index
    ordc = np.argsort(cand_safe, axis=1)
    cand_sorted = np.take_along_axis(cand_safe, ordc, axis=1)
    vals_sorted = np.take_along_axis(vals, ordc, axis=1)
    sel = np.argsort(-vals_sorted, axis=1, kind="stable")[:, :k]

    top_vals = np.take_along_axis(vals_sorted, sel, axis=1)
    top_idx = np.take_along_axis(cand_sorted, sel, axis=1)
    distances = (np.float32(1.0) - top_vals).astype(np.float32)
    indices = top_idx.astype(np.int32)
    return distances, indices


def kernel(queries, memory, k):
    queries_np = np.ascontiguousarray(np.asarray(queries, dtype=np.float32))
    memory_np = np.ascontiguousarray(np.asarray(memory, dtype=np.float32))
    k = int(np.asarray(k))

    mem_padded = np.zeros((M_PAD_TOTAL, D), dtype=np.float32)
    mem_padded[:M_TOTAL] = memory_np

    res = _run_device(queries_np, mem_padded)
    cm_all = [res.results[c]["cm"] for c in range(N_CORES)]
    return _host_topk(queries_np, memory_np, cm_all, k)


# revision 27
# speedup vs baseline: 1.3002x; 1.3002x over previous
"""Distributed k-NN retrieval (MemoryBank) on 8 Trainium2 NeuronCores.

Strategy (memory rows sharded 8 ways, queries replicated):
  Device (per core):
    - normalize its memory shard rows (1/max(|m|,eps)), cast bf16,
      DMA-transpose to [D, M] layout; cast+transpose queries (not normalized:
      a per-query positive scale never changes that query's ranking).
    - 32 query tiles x 26 matmul chunks (N=512) -> PSUM f32 sims.
    - max-accumulate drain: tensor_tensor(max)(acc, psum, acc) folds each
      chunk into a [128, 1024] bf16 accumulator (group u = stride-1024 family
      of 13 memory rows). One PSUM read per sims element - the hard floor.
    - ship [4096, 1024] bf16 group-max matrix per core to host.
  Host:
    - top-6 groups per query across all cores (top-3 groups provably contain
      the true top-3 values), rescore <=78 candidate rows exactly in fp32,
      emit top-k (distances = 1-sims, indices), ties -> lowest index.
"""

import functools

import numpy as np

# ---- hardcoded problem geometry (self-contained; do not read spec files) ----
NQ = 4096          # queries
D = 128            # feature dim
M_TOTAL = 100000   # memory rows
N_CORES = 8
M_SHARD = 13312    # padded per-core rows = 104*128 = 13*1024
M_PAD_TOTAL = M_SHARD * N_CORES
NQT = NQ // 128    # 32 query tiles
N_PS = 13          # psum tiles of 1024 (2 banks) per query tile
CM_W = 1024        # group-max width; group u = rows {1024*i + u}
N_MTILE = M_SHARD // 128  # 104
EPS = 1e-12

# number of top groups rescored on host (3 suffices in exact arithmetic;
# extra groups absorb bf16 rounding ties)
T_GROUPS = 6

# psum-tile routing: which engine drains each of the 13 psum tiles.
# 'D' = DVE reads PSUM directly (1x); 'A' = ScalarE cast-copies to SBUF,
# DVE pairs in bf16 at 2x. Each route has its own accumulator
# (cross-engine accumulation on one tile would serialize the engines).
# GpSimd has no TENSOR_TENSOR opcode on TRN2 (ISA engine check).
ROUTE_NAMES = "DA"
ROUTES = "ADAADAADAADAA"
assert len(ROUTES) == N_PS
ROUTE_CHUNKS = {r: [i for i, c in enumerate(ROUTES) if c == r]
                for r in ROUTE_NAMES}
N_ROUTES = len(ROUTE_NAMES)


@functools.lru_cache(maxsize=1)
def _build_nc():
    import concourse.mybir as mybir
    from concourse import bacc, tile

    f32 = mybir.dt.float32
    bf16 = mybir.dt.bfloat16
    AF = mybir.ActivationFunctionType
    MAX = mybir.AluOpType.max
    AX = mybir.AxisListType.X

    nc = bacc.Bacc("TRN2", target_bir_lowering=False, debug=False)

    mem_in = nc.dram_tensor("mem", [M_SHARD, D], f32, kind="ExternalInput")
    q_in = nc.dram_tensor("queries", [NQ, D], f32, kind="ExternalInput")
    id_in = nc.dram_tensor("ident", [128, 128], bf16, kind="ExternalInput")
    cm_out = nc.dram_tensor(
        "cm", [NQ, N_ROUTES * CM_W], bf16, kind="ExternalOutput")

    with tile.TileContext(nc) as tc:
        with (
            tc.tile_pool(name="const", bufs=1) as const_pool,
            tc.tile_pool(name="stage", bufs=1) as stage_pool,
            tc.tile_pool(name="prep", bufs=2) as prep_pool,
            tc.tile_pool(name="psum", bufs=3, space="PSUM") as psum_pool,
            tc.tile_pool(name="tpsum", bufs=2, space="PSUM") as tpsum_pool,
            tc.tile_pool(name="work", bufs=2) as work_pool,
        ):
            mT = const_pool.tile([128, M_SHARD], bf16, tag="mT")
            qT = const_pool.tile([128, NQ], bf16, tag="qT")
            ident = const_pool.tile([128, 128], bf16, tag="ident")
            nc.sync.dma_start(ident[:], id_in.ap())

            # ---------------- prep: queries -> qT (bf16, transposed) --------
            qstage = stage_pool.tile([128, NQT * D], f32, tag="qstage")
            nc.sync.dma_start(
                qstage[:].rearrange("p (t d) -> p t d", d=D),
                q_in.ap().rearrange("(t p) d -> p t d", p=128),
            )
            identf = const_pool.tile([128, 128], f32, tag="identf")
            nc.scalar.copy(identf[:], ident[:])
            for t in range(NQT):
                tp = tpsum_pool.tile([128, 128], f32, tag="tp")
                nc.tensor.transpose(
                    tp[:], qstage[:, t * D:(t + 1) * D], identf[:])
                nc.vector.tensor_copy(qT[:, t * 128:(t + 1) * 128], tp[:])

            # ---------------- prep: memory -> mT (normalized bf16, transposed)
            ss = const_pool.tile([128, N_MTILE], f32, tag="ss")
            N_PIECE = 4
            TPP = N_MTILE // N_PIECE  # 26 tiles per piece
            for piece in range(N_PIECE):
                mstage = stage_pool.tile([128, TPP * D], f32, tag=f"mstage{piece}")
                r0 = piece * TPP * 128
                nc.sync.dma_start(
                    mstage[:].rearrange("p (t d) -> p t d", d=D),
                    mem_in.ap()[r0:r0 + TPP * 128, :].rearrange(
                        "(t p) d -> p t d", p=128),
                )
                sq = prep_pool.tile([128, TPP * D], f32, tag="sq")
                nc.scalar.activation(sq[:], mstage[:], AF.Square)
                nc.vector.reduce_sum(
                    ss[:, piece * TPP:(piece + 1) * TPP],
                    sq[:].rearrange("p (t d) -> p t d", d=D),
                    axis=AX,
                )
                # normalized bf16 tiles + transpose into mT
                m_bf = prep_pool.tile([128, TPP * D], bf16, tag="m_bf")
                norm = prep_pool.tile([128, TPP], f32, tag="norm")
                scale = prep_pool.tile([128, TPP], f32, tag="scale")
                nc.scalar.activation(
                    norm[:], ss[:, piece * TPP:(piece + 1) * TPP], AF.Sqrt)
                nc.vector.tensor_scalar_max(norm[:], norm[:], EPS)
                nc.vector.reciprocal(scale[:], norm[:])
                for t in range(TPP):
                    nc.scalar.activation(
                        m_bf[:, t * D:(t + 1) * D],
                        mstage[:, t * D:(t + 1) * D],
                        AF.Copy,
                        scale=scale[:, t:t + 1],
                    )
                for t in range(TPP):
                    tg = piece * TPP + t
                    tp = tpsum_pool.tile([128, 128], bf16, tag="tp")
                    nc.tensor.transpose(tp[:], m_bf[:, t * D:(t + 1) * D], ident[:])
                    nc.vector.tensor_copy(mT[:, tg * 128:(tg + 1) * 128], tp[:])

            # ---------------- main: sims + routed max-accumulate drains -----
            # psum tile c covers mT cols [1024c, 1024c+1024); route r's acc
            # accumulates max over its chunk list; host merges the 3 accs.
            for qt in range(NQT):
                accs = {r: work_pool.tile([128, CM_W], bf16, tag=f"acc{r}",
                                          name=f"acc{r}")
                        for r in ROUTE_NAMES}
                seen = {r: False for r in ROUTE_NAMES}
                lhsT = qT[:, qt * 128:(qt + 1) * 128]
                for c in range(N_PS):
                    ps = psum_pool.tile([128, 1024], f32, tag="ps")
                    nc.tensor.matmul(
                        ps[:, 0:512], lhsT, mT[:, c * 1024:c * 1024 + 512],
                        start=True, stop=True,
                    )
                    nc.tensor.matmul(
                        ps[:, 512:1024], lhsT,
                        mT[:, c * 1024 + 512:(c + 1) * 1024],
                        start=True, stop=True,
                    )
                    r = ROUTES[c]
                    acc = accs[r]
                    if r == "D":
                        if not seen[r]:
                            nc.vector.tensor_copy(acc[:], ps[:])
                        else:
                            nc.vector.tensor_tensor(acc[:], ps[:], acc[:], op=MAX)
                    elif not seen[r]:
                        nc.scalar.copy(acc[:], ps[:])  # init: cast-copy to acc
                    else:
                        tmp = work_pool.tile([128, CM_W], bf16, tag=f"tmp{r}")
                        nc.scalar.copy(tmp[:], ps[:])
                        nc.vector.tensor_tensor(acc[:], tmp[:], acc[:], op=MAX)
                    seen[r] = True
                for ri, r in enumerate(ROUTE_NAMES):
                    nc.sync.dma_start(
                        cm_out.ap()[qt * 128:(qt + 1) * 128,
                                    ri * CM_W:(ri + 1) * CM_W],
                        accs[r][:],
                    )

    nc.compile()
    return nc


def _identity_bf16():
    import ml_dtypes

    return np.eye(128, dtype=ml_dtypes.bfloat16)


def _in_maps(queries_np, mem_padded):
    shards = mem_padded.reshape(N_CORES, M_SHARD, D)
    ident = _identity_bf16()
    return [
        {"mem": np.ascontiguousarray(shards[c]), "queries": queries_np,
         "ident": ident}
        for c in range(N_CORES)
    ]


def _run_device(queries_np, mem_padded, trace=False):
    from concourse import bass_utils

    nc = _build_nc()
    res = bass_utils.run_bass_kernel_spmd(
        nc, _in_maps(queries_np, mem_padded),
        core_ids=list(range(N_CORES)), trace=trace,
    )
    return res


def _host_topk(queries_np, memory_np, cm_all, k):
    import ml_dtypes  # noqa: F401  (cm arrives as bfloat16)

    nq = queries_np.shape[0]
    # [NQ, N_CORES * 3 * CM_W] routed group-max matrix
    cm = np.concatenate(
        [np.asarray(cm_all[c], dtype=np.float32) for c in range(N_CORES)], axis=1
    )
    t = min(T_GROUPS, cm.shape[1])
    top_groups = np.argpartition(-cm, t - 1, axis=1)[:, :t]  # [NQ, t]

    per_core = N_ROUTES * CM_W
    core = top_groups // per_core
    rem = top_groups % per_core
    ri = rem // CM_W
    u = rem % CM_W
    # route ri group u covers local rows {1024*c + u : c in ROUTE_CHUNKS[route]}
    max_members = max(len(v) for v in ROUTE_CHUNKS.values())
    chunk_arr = np.full((N_ROUTES, max_members), -1, dtype=np.int64)
    for j, r in enumerate(ROUTE_NAMES):
        chunk_arr[j, :len(ROUTE_CHUNKS[r])] = ROUTE_CHUNKS[r]
    chunks = chunk_arr[ri]                                 # [NQ, t, max_members]
    loc = CM_W * chunks + u[:, :, None]                    # [NQ, t, mm]
    cand = (core[:, :, None] * M_SHARD + loc).reshape(nq, -1)
    cand_pad_invalid = (chunks < 0).reshape(nq, -1)
    cand = np.where(cand_pad_invalid, M_PAD_TOTAL, cand)   # force invalid

    valid = cand < M_TOTAL
    cand_safe = np.where(valid, cand, 0)

    qn = queries_np / np.maximum(
        np.linalg.norm(queries_np, axis=1, keepdims=True), EPS)
    mc = memory_np[cand_safe]                             # [NQ, t*16, D]
    mc_n = np.linalg.norm(mc, axis=2, keepdims=True)
    mc = mc / np.maximum(mc_n, EPS)
    vals = np.einsum("qd,qcd->qc", qn.astype(np.float32), mc.astype(np.float32))
    vals = np.where(valid, vals, np.float32(-2.0))

    # sort candidates by index so a stable sort on -vals breaks ties by index
    ordc = np.argsort(cand_safe, axis=1)
    cand_sorted = np.take_along_axis(cand_safe, ordc, axis=1)
    vals_sorted = np.take_along_axis(vals, ordc, axis=1)
    sel = np.argsort(-vals_sorted, axis=1, kind="stable")[:, :k]

    top_vals = np.take_along_axis(vals_sorted, sel, axis=1)
    top_idx = np.take_along_axis(cand_sorted, sel, axis=1)
    distances = (np.float32(1.0) - top_vals).astype(np.float32)
    indices = top_idx.astype(np.int32)
    return distances, indices


def kernel(queries, memory, k):
    queries_np = np.ascontiguousarray(np.asarray(queries, dtype=np.float32))
    memory_np = np.ascontiguousarray(np.asarray(memory, dtype=np.float32))
    k = int(np.asarray(k))

    mem_padded = np.zeros((M_PAD_TOTAL, D), dtype=np.float32)
    mem_padded[:M_TOTAL] = memory_np

    res = _run_device(queries_np, mem_padded)
    cm_all = [res.results[c]["cm"] for c in range(N_CORES)]
    return _host_topk(queries_np, memory_np, cm_all, k)
